# revision 2
# baseline (speedup 1.0000x reference)
"""Mixture-of-Depths router kernel for 8 Trainium2 NeuronCores.

Reference computation (B=4, S=4096, D=4096, H=1024, k=S/2=2048):
    h = relu(x @ w1 + b1); scores = (h @ w2 + b2)[..., 0]
    topk_scores, topk_idx = top_k(scores, k)           # per row over S
    mask[rows, topk_idx] = True
    routing_weights[rows, sort(topk_idx)] = softmax(topk_scores)
    (note: weights are scrambled -- the j-th smallest selected index
     receives the softmax of the j-th LARGEST score)

Distribution: the 16384 (b, s) rows are sharded 2048/core for the MLP
scorer (dominant compute, ~17 GFLOP/core, fp16x3 split matmuls for fp32
accuracy).  Cores 2b and 2b+1 hold row b's score halves; a pairwise
AllGather gives both the full row, and each pair redundantly runs the
top-k/softmax/scatter phase for its row, so no core-dependent
addressing is needed.  Top-k is computed via exact ranks
(rank_s = #{u : z_u > z_s}, fp32-exact), the descending-sorted weight
table is built with one-hot permutation matmuls on the tensor engine,
and the scrambled scatter becomes a monotone gather through the
prefix-sum of the mask (gpsimd ap_gather).
"""
import numpy as np

import concourse.bacc as bacc
import concourse.tile as tile
import concourse.mybir as mybir
from concourse import bass_isa
from concourse.bass_utils import run_bass_kernel_spmd

F32 = mybir.dt.float32
F16 = mybir.dt.float16
U8 = mybir.dt.uint8
I16 = mybir.dt.int16
OP = mybir.AluOpType
AX = mybir.AxisListType
ACT = mybir.ActivationFunctionType

B, S, D, H = 4, 4096, 4096, 1024
K = S // 2                  # 2048 selected per row
NCORES = 8
ROWS_PER_CORE = 2048        # (b, s) rows of x per core
NST = ROWS_PER_CORE // 128  # 16 seq tiles per core
NDC = D // 128              # 32 contraction chunks
TAB = K + 128               # gather table size (zero slot at index K)

_CACHED = {}


def _build():
    nc = bacc.Bacc("TRN2", target_bir_lowering=False, debug=False,
                   num_devices=NCORES)
    xs_d = nc.dram_tensor("xs", [ROWS_PER_CORE, D], F32, kind="ExternalInput")
    w1_d = nc.dram_tensor("w1", [D, H], F32, kind="ExternalInput")
    b1_d = nc.dram_tensor("b1", [H], F32, kind="ExternalInput")
    w2_d = nc.dram_tensor("w2", [H, 1], F32, kind="ExternalInput")
    b2_d = nc.dram_tensor("b2", [1], F32, kind="ExternalInput")
    mask_d = nc.dram_tensor("mask_row", [S], U8, kind="ExternalOutput")
    rw_d = nc.dram_tensor("rw_row", [S], F32, kind="ExternalOutput")

    with tile.TileContext(nc) as tc:
        with (
            tc.tile_pool(name="keep", bufs=1) as keep,
            tc.tile_pool(name="dram", bufs=1, space="DRAM") as dram,
        ):
            # ---------------- constants ----------------
            w2rep = keep.tile([128, H], F32)
            nc.sync.dma_start(
                w2rep[:],
                w2_d.ap().rearrange("h o -> (h o)").unsqueeze(0).broadcast_to([128, H]))
            b2col = keep.tile([128, 1], F32)
            nc.sync.dma_start(b2col[:], b2_d.ap().unsqueeze(0).broadcast_to([128, 1]))
            b1sb = keep.tile([1, H], F32)
            nc.sync.dma_start(b1sb[:], b1_d.ap().unsqueeze(0))
            b1h = keep.tile([1, H], F16)
            b1l = keep.tile([1, H], F16)
            nc.vector.tensor_copy(b1h[:], b1sb[:])
            nc.vector.tensor_tensor(b1l[:], b1sb[:], b1h[:], OP.subtract)
            onesrow = keep.tile([1, 128], F16)
            nc.vector.memset(onesrow[:], 1.0)

            iotasq = keep.tile([128, 128], F32)   # value = f - p
            nc.gpsimd.iota(iotasq[:], [[1, 128]], base=0, channel_multiplier=-1,
                           allow_small_or_imprecise_dtypes=True)
            ident16 = keep.tile([128, 128], F16)  # PE transpose identity
            nc.vector.tensor_scalar(ident16[:], iotasq[:], 0.0, None, OP.is_equal)
            lstrict = keep.tile([128, 128], F16)  # [p, f] = 1 if f > p
            nc.vector.tensor_scalar(lstrict[:], iotasq[:], 0.0, None, OP.is_gt)
            onescol = keep.tile([128, 1], F16)
            nc.vector.memset(onescol[:], 1.0)
            scores_sb = keep.tile([128, NST], F32)

            # ---------------- phase 1: scores = mlp(x) ----------------
            with (
                tc.tile_pool(name="w1pool", bufs=1) as w1pool,
                tc.tile_pool(name="xpool", bufs=2) as xpool,
                tc.tile_pool(name="xtpool", bufs=1) as xtpool,
                tc.tile_pool(name="epi", bufs=1) as epi,
                tc.tile_pool(name="pmm", bufs=2, space="PSUM") as pmm,
                tc.tile_pool(name="ptp", bufs=4, space="PSUM") as ptp,
            ):
                w1h = w1pool.tile([128, NDC * H], F16)
                w1l = w1pool.tile([128, NDC * H], F16)
                for dc in range(NDC):
                    wtmp = xpool.tile([128, H], F32, tag="x32")
                    nc.sync.dma_start(wtmp[:], w1_d.ap()[dc * 128:(dc + 1) * 128, :])
                    hview = w1h[:, dc * H:(dc + 1) * H]
                    nc.vector.tensor_copy(hview, wtmp[:])
                    nc.vector.tensor_tensor(w1l[:, dc * H:(dc + 1) * H],
                                            wtmp[:], hview, OP.subtract)

                for st in range(NST):
                    rows = slice(st * 128, (st + 1) * 128)
                    xh = xtpool.tile([128, D], F16, tag="xh")
                    xl = xtpool.tile([128, D], F16, tag="xl")
                    for half in range(2):
                        cols = slice(half * (D // 2), (half + 1) * (D // 2))
                        x32 = xpool.tile([128, D // 2], F32, tag="x32")
                        nc.sync.dma_start(x32[:], xs_d.ap()[rows, cols])
                        nc.vector.tensor_copy(xh[:, cols], x32[:])
                        nc.vector.tensor_tensor(xl[:, cols], x32[:], xh[:, cols],
                                                OP.subtract)
                    # transpose 128x128 blocks: xT[d, s]
                    xhT = xtpool.tile([128, D], F16, tag="xhT")
                    xlT = xtpool.tile([128, D], F16, tag="xlT")
                    for dc in range(NDC):
                        blk = slice(dc * 128, (dc + 1) * 128)
                        for src, dst in ((xh, xhT), (xl, xlT)):
                            pt = ptp.tile([128, 128], F16, tag="ptp")
                            nc.tensor.transpose(pt[:], src[:, blk], ident16[:])
                            nc.vector.tensor_copy(dst[:, blk], pt[:])

                    hpsum = pmm.tile([128, H], F32, tag="hpsum")
                    for dc in range(NDC):
                        blk = slice(dc * 128, (dc + 1) * 128)
                        first = dc == 0
                        for nh in range(2):
                            ncols = slice(nh * 512, (nh + 1) * 512)
                            wb = slice(dc * H + nh * 512, dc * H + (nh + 1) * 512)
                            nc.tensor.matmul(hpsum[:, ncols], xhT[:, blk],
                                             w1h[:, wb], start=first, stop=False)
                            nc.tensor.matmul(hpsum[:, ncols], xhT[:, blk],
                                             w1l[:, wb], start=False, stop=False)
                            nc.tensor.matmul(hpsum[:, ncols], xlT[:, blk],
                                             w1h[:, wb], start=False, stop=False)
                    # bias b1 (zero in practice, honored exactly)
                    for nh in range(2):
                        ncols = slice(nh * 512, (nh + 1) * 512)
                        nc.tensor.matmul(hpsum[:, ncols], onesrow[:],
                                         b1h[:, ncols], start=False, stop=False)
                        nc.tensor.matmul(hpsum[:, ncols], onesrow[:],
                                         b1l[:, ncols], start=False,
                                         stop=True)
                    # scores[:, st] = sum(relu(h) * w2)
                    escr = epi.tile([128, H], F32, tag="escr")
                    nc.vector.scalar_tensor_tensor(
                        escr[:], hpsum[:], 0.0, w2rep[:], OP.max, OP.mult,
                        accum_out=scores_sb[:, st:st + 1])
                nc.vector.tensor_scalar(scores_sb[:], scores_sb[:], b2col[:],
                                        None, OP.add)

            # ---------------- phase 1.5: pairwise allgather ----------------
            bounce_in = dram.tile([ROWS_PER_CORE], F32)
            bounce_pair = dram.tile([S], F32)
            nc.sync.dma_start(
                bounce_in[:].rearrange("(st p) -> st p", st=NST, p=128).transpose([1, 0]),
                scores_sb[:])
            nc.gpsimd.collective_compute(
                "AllGather", OP.bypass,
                replica_groups=[[0, 1], [2, 3], [4, 5], [6, 7]],
                ins=[bounce_in[:].opt()],
                outs=[bounce_pair[:].opt()],
            )

            # ---------------- phase 2: topk mask + scrambled softmax -------
            with (
                tc.tile_pool(name="p2", bufs=1) as p2,
                tc.tile_pool(name="p2s", bufs=2) as p2s,
                tc.tile_pool(name="pp2", bufs=2, space="PSUM") as pp2,
            ):
                iotaF = p2.tile([128, K], F32)   # 0..K-1 along free dim
                nc.gpsimd.iota(iotaF[:], [[1, K]], base=0, channel_multiplier=0,
                               allow_small_or_imprecise_dtypes=True)
                zrow = bounce_pair
                zB = p2.tile([128, 32], F32)     # z[128t + p] at [p, t]
                nc.sync.dma_start(
                    zB[:], zrow[:].rearrange("(t p) -> p t", t=32, p=128))
                zrep = p2.tile([128, S], F32)
                nc.sync.dma_start(
                    zrep[:], zrow[:].unsqueeze(0).broadcast_to([128, S]))

                # exact descending ranks: rank_s = #{u : z_u > z_s}
                ranksB = p2.tile([128, 32], F32)
                for t in range(32):
                    cscr = p2s.tile([128, S], mybir.dt.bfloat16, tag="cscr")
                    nc.vector.tensor_scalar(cscr[:], zrep[:], zB[:, t:t + 1],
                                            0.0, OP.is_gt, op1=OP.add,
                                            accum_out=ranksB[:, t:t + 1])

                maskf = p2.tile([128, 32], F32)
                nc.vector.tensor_scalar(maskf[:], ranksB[:], float(K), None,
                                        OP.is_lt)
                masku8 = p2.tile([128, 32], U8)
                nc.vector.tensor_copy(masku8[:], maskf[:])
                nc.sync.dma_start(
                    mask_d.ap().rearrange("(t p) -> p t", t=32, p=128), masku8[:])
                maskh = p2.tile([128, 32], F16)
                nc.vector.tensor_copy(maskh[:], maskf[:])

                # exclusive prefix sum of mask via triangular matmuls
                psPS = pp2.tile([128, 32], F32, tag="psPS")
                nc.tensor.matmul(psPS[:], lstrict[:], maskh[:], start=True,
                                 stop=False)
                csPS = pp2.tile([1, 32], F32, tag="csPS")
                nc.tensor.matmul(csPS[:], onescol[:], maskh[:], start=True,
                                 stop=True)
                cs = p2.tile([1, 32], F32)
                nc.vector.tensor_copy(cs[:], csPS[:])
                zero32 = p2.tile([1, 32], F32)
                nc.vector.memset(zero32[:], 0.0)
                incl = p2.tile([1, 32], F32)
                nc.vector.tensor_tensor_scan(incl[:], cs[:], zero32[:], 0.0,
                                             OP.add, OP.add)
                excl = p2.tile([1, 32], F16)
                nc.vector.tensor_tensor(excl[:], incl[:], cs[:], OP.subtract)
                nc.tensor.matmul(psPS[:], onesrow[:], excl[:], start=False,
                                 stop=True)
                psB = p2.tile([128, 32], F32)
                nc.vector.tensor_copy(psB[:], psPS[:])

                # softmax pieces: M = global max, E = exp(z - M), Z = sum(E*mask)
                zmax = p2.tile([128, 1], F32)
                nc.vector.tensor_reduce(zmax[:], zB[:], axis=AX.X, op=OP.max)
                Mcol = p2.tile([128, 1], F32)
                nc.gpsimd.partition_all_reduce(Mcol[:], zmax[:], channels=128,
                                               reduce_op=bass_isa.ReduceOp.max)
                negM = p2.tile([128, 1], F32)
                nc.vector.tensor_scalar(negM[:], Mcol[:], -1.0, None, OP.mult)
                Ef = p2.tile([128, 32], F32)
                nc.scalar.activation(Ef[:], zB[:], ACT.Exp, bias=negM[:])
                Emask = p2.tile([128, 32], F32)
                Zpart = p2.tile([128, 1], F32)
                nc.vector.scalar_tensor_tensor(Emask[:], Ef[:], 0.0, maskf[:],
                                               OP.add, OP.mult,
                                               accum_out=Zpart[:])
                Zcol = p2.tile([128, 1], F32)
                nc.gpsimd.partition_all_reduce(Zcol[:], Zpart[:], channels=128,
                                               reduce_op=bass_isa.ReduceOp.add)
                rZ = p2.tile([128, 1], F32)
                nc.vector.reciprocal(rZ[:], Zcol[:])

                # payload columns (E_s, 1) per s-chunk, fp16
                pay = p2.tile([128, 64], F16)
                nc.vector.memset(pay[:], 1.0)
                nc.vector.tensor_copy(
                    pay[:].rearrange("p (t two) -> p t two", t=32, two=2)[:, :, 0],
                    Ef[:])

                # permutation via one-hot matmuls: table[r] = (E_(r), count_r)
                tabPS = pp2.tile([128, 32], F32, tag="tabPS")
                for t in range(32):
                    oh = p2s.tile([128, K], F16, tag="oh")
                    nc.vector.tensor_scalar(oh[:], iotaF[:], ranksB[:, t:t + 1],
                                            None, OP.is_equal)
                    for rc in range(16):
                        nc.tensor.matmul(
                            tabPS[:, 2 * rc:2 * rc + 2],
                            oh[:, rc * 128:(rc + 1) * 128],
                            pay[:, 2 * t:2 * t + 2],
                            start=(t == 0), stop=(t == 31))
                tabsb = p2.tile([128, 32], F32)
                nc.vector.tensor_copy(tabsb[:], tabPS[:])
                tabv = tabsb[:].rearrange("p (rc two) -> p rc two", rc=16, two=2)
                sortE = p2.tile([128, 16], F32)
                cnt = p2.tile([128, 16], F32)
                nc.vector.tensor_copy(sortE[:], tabv[:, :, 0])
                nc.vector.tensor_copy(cnt[:], tabv[:, :, 1])

                # D = E/(max(cnt,1) * Z);  b = cnt > 0
                cmax = p2.tile([128, 16], F32)
                nc.vector.tensor_scalar(cmax[:], cnt[:], 1.0, None, OP.max)
                crec = p2.tile([128, 16], F32)
                nc.vector.reciprocal(crec[:], cmax[:])
                Dt = p2.tile([128, 16], F32)
                nc.vector.tensor_tensor(Dt[:], sortE[:], crec[:], OP.mult)
                Dv = p2.tile([128, 16], F32)
                nc.vector.tensor_scalar(Dv[:], Dt[:], rZ[:], None, OP.mult)
                bv = p2.tile([128, 16], F32)
                nc.vector.tensor_scalar(bv[:], cnt[:], 0.0, None, OP.is_gt)

                # round-trip to [1, K] layout for the backfill scan
                dD = dram.tile([K], F32)
                dB = dram.tile([K], F32)
                nc.sync.dma_start(
                    dD[:].rearrange("(rc m) -> m rc", rc=16, m=128), Dv[:])
                nc.sync.dma_start(
                    dB[:].rearrange("(rc m) -> m rc", rc=16, m=128), bv[:])
                Drow = p2.tile([1, K], F32)
                brow = p2.tile([1, K], F32)
                nc.sync.dma_start(Drow[:], dD[:].unsqueeze(0))
                nc.sync.dma_start(brow[:], dB[:].unsqueeze(0))
                onemb = p2.tile([1, K], F32)
                nc.vector.tensor_scalar(onemb[:], brow[:], -1.0, 1.0, OP.mult,
                                        op1=OP.add)
                wrow = p2.tile([1, K], F32)
                nc.vector.tensor_tensor_scan(wrow[:], onemb[:], Drow[:], 0.0,
                                             OP.mult, OP.add)

                # replicated gather table with zero slot at K
                dT = dram.tile([TAB], F32)
                zpad = p2.tile([1, TAB - K], F32)
                nc.vector.memset(zpad[:], 0.0)
                nc.sync.dma_start(dT[:][0:K].unsqueeze(0), wrow[:])
                nc.sync.dma_start(dT[:][K:TAB].unsqueeze(0), zpad[:])
                tabRep = p2.tile([128, TAB], F32)
                nc.sync.dma_start(tabRep[:],
                                  dT[:].unsqueeze(0).broadcast_to([128, TAB]))

                # idx = mask ? ps : K   (int16, wrapped layout for ap_gather)
                a1 = p2.tile([128, 32], F32)
                nc.vector.tensor_scalar(a1[:], psB[:], -float(K), None, OP.add)
                a2 = p2.tile([128, 32], F32)
                nc.vector.tensor_tensor(a2[:], a1[:], maskf[:], OP.mult)
                idxf = p2.tile([128, 32], F32)
                nc.vector.tensor_scalar(idxf[:], a2[:], float(K), None, OP.add)
                idx16 = p2.tile([128, 32], I16)
                nc.vector.tensor_copy(idx16[:], idxf[:])
                dI = dram.tile([S], I16)
                nc.sync.dma_start(
                    dI[:].rearrange("(t p) -> p t", t=32, p=128), idx16[:])
                idxW = p2.tile([128, 32], I16)
                for g in range(8):
                    nc.sync.dma_start(
                        idxW[16 * g:16 * (g + 1), :],
                        dI[:][512 * g:512 * (g + 1)]
                        .rearrange("(f m) -> f m", f=32, m=16).transpose([1, 0]))

                gout = p2.tile([128, 512], F32)
                nc.gpsimd.ap_gather(gout[:], tabRep[:], idxW[:], channels=128,
                                    num_elems=TAB, d=1, num_idxs=512)
                nc.sync.dma_start(
                    rw_d.ap().rearrange("(g f) -> g f", g=8, f=512),
                    gout[:].rearrange("(g m) f -> g m f", g=8, m=16)[:, 0, :])

    nc.finalize()
    return nc


def _get_nc():
    if "nc" not in _CACHED:
        _CACHED["nc"] = _build()
    return _CACHED["nc"]


def kernel(x, w1, b1, w2, b2):
    x = np.ascontiguousarray(np.asarray(x, dtype=np.float32))
    w1 = np.ascontiguousarray(np.asarray(w1, dtype=np.float32))
    b1 = np.ascontiguousarray(np.asarray(b1, dtype=np.float32))
    w2 = np.ascontiguousarray(np.asarray(w2, dtype=np.float32))
    b2 = np.ascontiguousarray(np.asarray(b2, dtype=np.float32))
    xf = x.reshape(B * S, D)

    nc = _get_nc()
    in_maps = [
        {
            "xs": np.ascontiguousarray(
                xf[c * ROWS_PER_CORE:(c + 1) * ROWS_PER_CORE]),
            "w1": w1, "b1": b1, "w2": w2, "b2": b2,
        }
        for c in range(NCORES)
    ]
    res = run_bass_kernel_spmd(nc, in_maps, core_ids=list(range(NCORES)))
    mask = np.stack([res.results[2 * b]["mask_row"] for b in range(B)])
    rw = np.stack([res.results[2 * b]["rw_row"] for b in range(B)])
    return mask.astype(bool), rw.astype(np.float32)


# revision 4
# speedup vs baseline: 1.1883x; 1.1883x over previous
"""Mixture-of-Depths router kernel for 8 Trainium2 NeuronCores.

Reference computation (B=4, S=4096, D=4096, H=1024, k=S/2=2048):
    h = relu(x @ w1 + b1); scores = (h @ w2 + b2)[..., 0]
    topk_scores, topk_idx = top_k(scores, k)           # per row over S
    mask[rows, topk_idx] = True
    routing_weights[rows, sort(topk_idx)] = softmax(topk_scores)
    (note: weights are scrambled -- the j-th smallest selected index
     receives the softmax of the j-th LARGEST score)

Distribution: the 16384 (b, s) rows are sharded 2048/core for the MLP
scorer (dominant compute, ~17 GFLOP/core, fp16x3 split matmuls for fp32
accuracy).  Cores 2b and 2b+1 hold row b's score halves; a pairwise
AllGather gives both the full row, and each pair redundantly runs the
top-k/softmax/scatter phase for its row, so no core-dependent
addressing is needed.  Top-k is computed via exact ranks
(rank_s = #{u : z_u > z_s}, fp32-exact), the descending-sorted weight
table is built with one-hot permutation matmuls on the tensor engine,
and the scrambled scatter becomes a monotone gather through the
prefix-sum of the mask (gpsimd ap_gather).
"""
import numpy as np

import concourse.bacc as bacc
import concourse.tile as tile
import concourse.mybir as mybir
from concourse import bass_isa
from concourse.bass_utils import run_bass_kernel_spmd

F32 = mybir.dt.float32
F16 = mybir.dt.float16
U8 = mybir.dt.uint8
I16 = mybir.dt.int16
OP = mybir.AluOpType
AX = mybir.AxisListType
ACT = mybir.ActivationFunctionType

B, S, D, H = 4, 4096, 4096, 1024
K = S // 2                  # 2048 selected per row
NCORES = 8
ROWS_PER_CORE = 2048        # (b, s) rows of x per core
NST = ROWS_PER_CORE // 128  # 16 seq tiles per core
NDC = D // 128              # 32 contraction chunks
TAB = K + 128               # gather table size (zero slot at index K)

_CACHED = {}


def _build():
    nc = bacc.Bacc("TRN2", target_bir_lowering=False, debug=False,
                   num_devices=NCORES)
    xs_d = nc.dram_tensor("xs", [ROWS_PER_CORE, D], F32, kind="ExternalInput")
    w1_d = nc.dram_tensor("w1", [D, H], F32, kind="ExternalInput")
    b1_d = nc.dram_tensor("b1", [H], F32, kind="ExternalInput")
    w2_d = nc.dram_tensor("w2", [H, 1], F32, kind="ExternalInput")
    b2_d = nc.dram_tensor("b2", [1], F32, kind="ExternalInput")
    mask_d = nc.dram_tensor("mask_row", [S], U8, kind="ExternalOutput")
    rw_d = nc.dram_tensor("rw_row", [S], F32, kind="ExternalOutput")
    dbgz_d = nc.dram_tensor("dbg_z", [S], F32, kind="ExternalOutput")
    dbgr_d = nc.dram_tensor("dbg_ranks", [S], F32, kind="ExternalOutput")
    dbgc_d = nc.dram_tensor("dbg_cnt", [K], F32, kind="ExternalOutput")
    dbgp_d = nc.dram_tensor("dbg_ps", [S], F32, kind="ExternalOutput")
    dbgt_d = nc.dram_tensor("dbg_tab", [K], F32, kind="ExternalOutput")

    with tile.TileContext(nc) as tc:
        with (
            tc.tile_pool(name="keep", bufs=1) as keep,
            tc.tile_pool(name="dram", bufs=1, space="DRAM") as dram,
        ):
            # ---------------- constants ----------------
            w2rep = keep.tile([128, H], F32)
            nc.sync.dma_start(
                w2rep[:],
                w2_d.ap().rearrange("h o -> (h o)").unsqueeze(0).broadcast_to([128, H]))
            b2col = keep.tile([128, 1], F32)
            nc.sync.dma_start(b2col[:], b2_d.ap().unsqueeze(0).broadcast_to([128, 1]))
            b1sb = keep.tile([1, H], F32)
            nc.sync.dma_start(b1sb[:], b1_d.ap().unsqueeze(0))
            b1h = keep.tile([1, H], F16)
            b1l = keep.tile([1, H], F16)
            nc.vector.tensor_copy(b1h[:], b1sb[:])
            nc.vector.tensor_tensor(b1l[:], b1sb[:], b1h[:], OP.subtract)
            onesrow = keep.tile([1, 128], F16)
            nc.vector.memset(onesrow[:], 1.0)

            iotasq = keep.tile([128, 128], F32)   # value = f - p
            nc.gpsimd.iota(iotasq[:], [[1, 128]], base=0, channel_multiplier=-1,
                           allow_small_or_imprecise_dtypes=True)
            ident16 = keep.tile([128, 128], F16)  # PE transpose identity
            nc.vector.tensor_scalar(ident16[:], iotasq[:], 0.0, None, OP.is_equal)
            lstrict = keep.tile([128, 128], F16)  # [p, f] = 1 if f > p
            nc.vector.tensor_scalar(lstrict[:], iotasq[:], 0.0, None, OP.is_gt)
            onescol = keep.tile([128, 1], F16)
            nc.vector.memset(onescol[:], 1.0)
            scores_sb = keep.tile([128, NST], F32)

            # ---------------- phase 1: scores = mlp(x) ----------------
            with (
                tc.tile_pool(name="w1pool", bufs=1) as w1pool,
                tc.tile_pool(name="xpool", bufs=2) as xpool,
                tc.tile_pool(name="xtpool", bufs=1) as xtpool,
                tc.tile_pool(name="epi", bufs=1) as epi,
                tc.tile_pool(name="pmm", bufs=2, space="PSUM") as pmm,
                tc.tile_pool(name="ptp", bufs=4, space="PSUM") as ptp,
            ):
                w1h = w1pool.tile([128, NDC * H], F16)
                w1l = w1pool.tile([128, NDC * H], F16)
                for dc in range(NDC):
                    wtmp = xpool.tile([128, H], F32, tag="x32")
                    nc.sync.dma_start(wtmp[:], w1_d.ap()[dc * 128:(dc + 1) * 128, :])
                    hview = w1h[:, dc * H:(dc + 1) * H]
                    nc.vector.tensor_copy(hview, wtmp[:])
                    nc.vector.tensor_tensor(w1l[:, dc * H:(dc + 1) * H],
                                            wtmp[:], hview, OP.subtract)

                for st in range(NST):
                    rows = slice(st * 128, (st + 1) * 128)
                    xh = xtpool.tile([128, D], F16, tag="xh")
                    xl = xtpool.tile([128, D], F16, tag="xl")
                    for half in range(2):
                        cols = slice(half * (D // 2), (half + 1) * (D // 2))
                        x32 = xpool.tile([128, D // 2], F32, tag="x32")
                        nc.sync.dma_start(x32[:], xs_d.ap()[rows, cols])
                        nc.vector.tensor_copy(xh[:, cols], x32[:])
                        nc.vector.tensor_tensor(xl[:, cols], x32[:], xh[:, cols],
                                                OP.subtract)
                    # transpose 128x128 blocks: xT[d, s]
                    xhT = xtpool.tile([128, D], F16, tag="xhT")
                    xlT = xtpool.tile([128, D], F16, tag="xlT")
                    for dc in range(NDC):
                        blk = slice(dc * 128, (dc + 1) * 128)
                        for src, dst in ((xh, xhT), (xl, xlT)):
                            pt = ptp.tile([128, 128], F16, tag="ptp")
                            nc.tensor.transpose(pt[:], src[:, blk], ident16[:])
                            nc.vector.tensor_copy(dst[:, blk], pt[:])

                    hpsum = pmm.tile([128, H], F32, tag="hpsum")
                    for dc in range(NDC):
                        blk = slice(dc * 128, (dc + 1) * 128)
                        first = dc == 0
                        for nh in range(2):
                            ncols = slice(nh * 512, (nh + 1) * 512)
                            wb = slice(dc * H + nh * 512, dc * H + (nh + 1) * 512)
                            nc.tensor.matmul(hpsum[:, ncols], xhT[:, blk],
                                             w1h[:, wb], start=first, stop=False)
                            nc.tensor.matmul(hpsum[:, ncols], xhT[:, blk],
                                             w1l[:, wb], start=False, stop=False)
                            nc.tensor.matmul(hpsum[:, ncols], xlT[:, blk],
                                             w1h[:, wb], start=False, stop=False)
                    # bias b1 (zero in practice, honored exactly)
                    for nh in range(2):
                        ncols = slice(nh * 512, (nh + 1) * 512)
                        nc.tensor.matmul(hpsum[:, ncols], onesrow[:],
                                         b1h[:, ncols], start=False, stop=False)
                        nc.tensor.matmul(hpsum[:, ncols], onesrow[:],
                                         b1l[:, ncols], start=False,
                                         stop=True)
                    # scores[:, st] = sum(relu(h) * w2)
                    escr = epi.tile([128, H], F32, tag="escr")
                    nc.vector.scalar_tensor_tensor(
                        escr[:], hpsum[:], 0.0, w2rep[:], OP.max, OP.mult,
                        accum_out=scores_sb[:, st:st + 1])
                nc.vector.tensor_scalar(scores_sb[:], scores_sb[:], b2col[:],
                                        None, OP.add)

            # ---------------- phase 1.5: pairwise allgather ----------------
            bounce_in = dram.tile([ROWS_PER_CORE], F32)
            bounce_pair = dram.tile([S], F32)
            nc.sync.dma_start(
                bounce_in[:].rearrange("(st p) -> st p", st=NST, p=128).transpose([1, 0]),
                scores_sb[:])
            nc.gpsimd.collective_compute(
                "AllGather", OP.bypass,
                replica_groups=[[0, 1], [2, 3], [4, 5], [6, 7]],
                ins=[bounce_in[:].opt()],
                outs=[bounce_pair[:].opt()],
            )

            # ---------------- phase 2: topk mask + scrambled softmax -------
            with (
                tc.tile_pool(name="p2", bufs=1) as p2,
                tc.tile_pool(name="p2s", bufs=2) as p2s,
                tc.tile_pool(name="pp2", bufs=2, space="PSUM") as pp2,
            ):
                iotaF = p2.tile([128, K], F32)   # 0..K-1 along free dim
                nc.gpsimd.iota(iotaF[:], [[1, K]], base=0, channel_multiplier=0,
                               allow_small_or_imprecise_dtypes=True)
                zrow = bounce_pair
                zB = p2.tile([128, 32], F32)     # z[128t + p] at [p, t]
                nc.sync.dma_start(
                    zB[:], zrow[:].rearrange("(t p) -> p t", t=32, p=128))
                zrep = p2.tile([128, S], F32)
                nc.sync.dma_start(
                    zrep[:], zrow[:].unsqueeze(0).broadcast_to([128, S]))

                # exact descending ranks: rank_s = #{u : z_u > z_s}
                nc.sync.dma_start(dbgz_d.ap().rearrange("(t p) -> p t", t=32, p=128), zB[:])
                ranksB = p2.tile([128, 32], F32)
                for t in range(32):
                    cscr = p2s.tile([128, S], mybir.dt.bfloat16, tag="cscr")
                    nc.vector.tensor_scalar(cscr[:], zrep[:], zB[:, t:t + 1],
                                            0.0, OP.is_gt, op1=OP.add,
                                            accum_out=ranksB[:, t:t + 1])

                nc.sync.dma_start(dbgr_d.ap().rearrange("(t p) -> p t", t=32, p=128), ranksB[:])
                maskf = p2.tile([128, 32], F32)
                nc.vector.tensor_scalar(maskf[:], ranksB[:], float(K), None,
                                        OP.is_lt)
                masku8 = p2.tile([128, 32], U8)
                nc.vector.tensor_copy(masku8[:], maskf[:])
                nc.sync.dma_start(
                    mask_d.ap().rearrange("(t p) -> p t", t=32, p=128), masku8[:])
                maskh = p2.tile([128, 32], F16)
                nc.vector.tensor_copy(maskh[:], maskf[:])

                # exclusive prefix sum of mask via triangular matmuls
                psPS = pp2.tile([128, 32], F32, tag="psPS")
                nc.tensor.matmul(psPS[:], lstrict[:], maskh[:], start=True,
                                 stop=False)
                csPS = pp2.tile([1, 32], F32, tag="csPS")
                nc.tensor.matmul(csPS[:], onescol[:], maskh[:], start=True,
                                 stop=True)
                cs = p2.tile([1, 32], F32)
                nc.vector.tensor_copy(cs[:], csPS[:])
                zero32 = p2.tile([1, 32], F32)
                nc.vector.memset(zero32[:], 0.0)
                incl = p2.tile([1, 32], F32)
                nc.vector.tensor_tensor_scan(incl[:], cs[:], zero32[:], 0.0,
                                             OP.add, OP.add)
                excl = p2.tile([1, 32], F16)
                nc.vector.tensor_tensor(excl[:], incl[:], cs[:], OP.subtract)
                nc.tensor.matmul(psPS[:], onesrow[:], excl[:], start=False,
                                 stop=True)
                psB = p2.tile([128, 32], F32)
                nc.vector.tensor_copy(psB[:], psPS[:])

                # softmax pieces: M = global max, E = exp(z - M), Z = sum(E*mask)
                zmax = p2.tile([128, 1], F32)
                nc.vector.tensor_reduce(zmax[:], zB[:], axis=AX.X, op=OP.max)
                Mcol = p2.tile([128, 1], F32)
                nc.gpsimd.partition_all_reduce(Mcol[:], zmax[:], channels=128,
                                               reduce_op=bass_isa.ReduceOp.max)
                negM = p2.tile([128, 1], F32)
                nc.vector.tensor_scalar(negM[:], Mcol[:], -1.0, None, OP.mult)
                Ef = p2.tile([128, 32], F32)
                nc.scalar.activation(Ef[:], zB[:], ACT.Exp, bias=negM[:])
                Emask = p2.tile([128, 32], F32)
                Zpart = p2.tile([128, 1], F32)
                nc.vector.scalar_tensor_tensor(Emask[:], Ef[:], 0.0, maskf[:],
                                               OP.add, OP.mult,
                                               accum_out=Zpart[:])
                Zcol = p2.tile([128, 1], F32)
                nc.gpsimd.partition_all_reduce(Zcol[:], Zpart[:], channels=128,
                                               reduce_op=bass_isa.ReduceOp.add)
                rZ = p2.tile([128, 1], F32)
                nc.vector.reciprocal(rZ[:], Zcol[:])

                # payload columns (E_s, 1) per s-chunk, fp16
                pay = p2.tile([128, 64], F16)
                nc.vector.memset(pay[:], 1.0)
                nc.vector.tensor_copy(
                    pay[:].rearrange("p (t two) -> p t two", t=32, two=2)[:, :, 0],
                    Ef[:])

                # permutation via one-hot matmuls: table[r] = (E_(r), count_r)
                # (each t is a self-contained start/stop set into a fresh PSUM
                # tile -- interleaved accumulation groups in one bank clobber
                # each other's has_written state -- then accumulate on DVE)
                tabsb = p2.tile([128, 32], F32)
                nc.vector.memset(tabsb[:], 0.0)
                for t in range(32):
                    oh = p2s.tile([128, K], F16, tag="oh")
                    nc.vector.tensor_scalar(oh[:], iotaF[:], ranksB[:, t:t + 1],
                                            None, OP.is_equal)
                    tps = pp2.tile([128, 32], F32, tag="tabPS")
                    for rc in range(16):
                        nc.tensor.matmul(
                            tps[:, 2 * rc:2 * rc + 2],
                            oh[:, rc * 128:(rc + 1) * 128],
                            pay[:, 2 * t:2 * t + 2],
                            start=True, stop=True)
                    nc.vector.tensor_tensor(tabsb[:], tabsb[:], tps[:], OP.add)
                tabv = tabsb[:].rearrange("p (rc two) -> p rc two", rc=16, two=2)
                sortE = p2.tile([128, 16], F32)
                cnt = p2.tile([128, 16], F32)
                nc.vector.tensor_copy(sortE[:], tabv[:, :, 0])
                nc.vector.tensor_copy(cnt[:], tabv[:, :, 1])

                nc.sync.dma_start(dbgc_d.ap().rearrange("(rc m) -> m rc", rc=16, m=128), cnt[:])
                # D = E/(max(cnt,1) * Z);  b = cnt > 0
                cmax = p2.tile([128, 16], F32)
                nc.vector.tensor_scalar(cmax[:], cnt[:], 1.0, None, OP.max)
                crec = p2.tile([128, 16], F32)
                nc.vector.reciprocal(crec[:], cmax[:])
                Dt = p2.tile([128, 16], F32)
                nc.vector.tensor_tensor(Dt[:], sortE[:], crec[:], OP.mult)
                Dv = p2.tile([128, 16], F32)
                nc.vector.tensor_scalar(Dv[:], Dt[:], rZ[:], None, OP.mult)
                bv = p2.tile([128, 16], F32)
                nc.vector.tensor_scalar(bv[:], cnt[:], 0.0, None, OP.is_gt)

                # round-trip to [1, K] layout for the backfill scan
                dD = dram.tile([K], F32)
                dB = dram.tile([K], F32)
                nc.sync.dma_start(
                    dD[:].rearrange("(rc m) -> m rc", rc=16, m=128), Dv[:])
                nc.sync.dma_start(
                    dB[:].rearrange("(rc m) -> m rc", rc=16, m=128), bv[:])
                Drow = p2.tile([1, K], F32)
                brow = p2.tile([1, K], F32)
                nc.sync.dma_start(Drow[:], dD[:].unsqueeze(0))
                nc.sync.dma_start(brow[:], dB[:].unsqueeze(0))
                onemb = p2.tile([1, K], F32)
                nc.vector.tensor_scalar(onemb[:], brow[:], -1.0, 1.0, OP.mult,
                                        op1=OP.add)
                wrow = p2.tile([1, K], F32)
                nc.vector.tensor_tensor_scan(wrow[:], onemb[:], Drow[:], 0.0,
                                             OP.mult, OP.add)

                # replicated gather table with zero slot at K
                dT = dram.tile([TAB], F32)
                zpad = p2.tile([1, TAB - K], F32)
                nc.vector.memset(zpad[:], 0.0)
                nc.sync.dma_start(dT[:][0:K].unsqueeze(0), wrow[:])
                nc.sync.dma_start(dT[:][K:TAB].unsqueeze(0), zpad[:])
                nc.sync.dma_start(dbgt_d.ap().unsqueeze(0), wrow[:])
                tabRep = p2.tile([128, TAB], F32)
                nc.sync.dma_start(tabRep[:],
                                  dT[:].unsqueeze(0).broadcast_to([128, TAB]))

                # idx = mask ? ps : K   (int16, wrapped layout for ap_gather)
                a1 = p2.tile([128, 32], F32)
                nc.vector.tensor_scalar(a1[:], psB[:], -float(K), None, OP.add)
                a2 = p2.tile([128, 32], F32)
                nc.vector.tensor_tensor(a2[:], a1[:], maskf[:], OP.mult)
                idxf = p2.tile([128, 32], F32)
                nc.vector.tensor_scalar(idxf[:], a2[:], float(K), None, OP.add)
                nc.sync.dma_start(dbgp_d.ap().rearrange("(t p) -> p t", t=32, p=128), psB[:])
                idx16 = p2.tile([128, 32], I16)
                nc.vector.tensor_copy(idx16[:], idxf[:])
                dI = dram.tile([S], I16)
                nc.sync.dma_start(
                    dI[:].rearrange("(t p) -> p t", t=32, p=128), idx16[:])
                idxW = p2.tile([128, 32], I16)
                for g in range(8):
                    nc.sync.dma_start(
                        idxW[16 * g:16 * (g + 1), :],
                        dI[:][512 * g:512 * (g + 1)]
                        .rearrange("(f m) -> f m", f=32, m=16).transpose([1, 0]))

                gout = p2.tile([128, 512], F32)
                nc.gpsimd.ap_gather(gout[:], tabRep[:], idxW[:], channels=128,
                                    num_elems=TAB, d=1, num_idxs=512)
                nc.sync.dma_start(
                    rw_d.ap().rearrange("(g f) -> g f", g=8, f=512),
                    gout[:].rearrange("(g m) f -> g m f", g=8, m=16)[:, 0, :])

    nc.finalize()
    return nc


def _get_nc():
    if "nc" not in _CACHED:
        _CACHED["nc"] = _build()
    return _CACHED["nc"]


def kernel(x, w1, b1, w2, b2):
    x = np.ascontiguousarray(np.asarray(x, dtype=np.float32))
    w1 = np.ascontiguousarray(np.asarray(w1, dtype=np.float32))
    b1 = np.ascontiguousarray(np.asarray(b1, dtype=np.float32))
    w2 = np.ascontiguousarray(np.asarray(w2, dtype=np.float32))
    b2 = np.ascontiguousarray(np.asarray(b2, dtype=np.float32))
    xf = x.reshape(B * S, D)

    nc = _get_nc()
    in_maps = [
        {
            "xs": np.ascontiguousarray(
                xf[c * ROWS_PER_CORE:(c + 1) * ROWS_PER_CORE]),
            "w1": w1, "b1": b1, "w2": w2, "b2": b2,
        }
        for c in range(NCORES)
    ]
    res = run_bass_kernel_spmd(nc, in_maps, core_ids=list(range(NCORES)))
    mask = np.stack([res.results[2 * b]["mask_row"] for b in range(B)])
    rw = np.stack([res.results[2 * b]["rw_row"] for b in range(B)])
    return mask.astype(bool), rw.astype(np.float32)


# revision 6
# speedup vs baseline: 80.5432x; 67.7798x over previous
"""Mixture-of-Depths router kernel for 8 Trainium2 NeuronCores.

Reference computation (B=4, S=4096, D=4096, H=1024, k=S/2=2048):
    h = relu(x @ w1 + b1); scores = (h @ w2 + b2)[..., 0]
    topk_scores, topk_idx = top_k(scores, k)           # per row over S
    mask[rows, topk_idx] = True
    routing_weights[rows, sort(topk_idx)] = softmax(topk_scores)
    (note: weights are scrambled -- the j-th smallest selected index
     receives the softmax of the j-th LARGEST score)

Distribution: the 16384 (b, s) rows are sharded 2048/core for the MLP
scorer (dominant compute, ~17 GFLOP/core, fp16x3 split matmuls for fp32
accuracy).  Cores 2b and 2b+1 hold row b's score halves; a pairwise
AllGather gives both the full row, and each pair redundantly runs the
top-k/softmax/scatter phase for its row, so no core-dependent
addressing is needed.  Top-k is computed via exact ranks
(rank_s = #{u : z_u > z_s}, fp32-exact), the descending-sorted weight
table is built with one-hot permutation matmuls on the tensor engine,
and the scrambled scatter becomes a monotone gather through the
prefix-sum of the mask (gpsimd ap_gather).
"""
import numpy as np

import concourse.bacc as bacc
import concourse.tile as tile
import concourse.mybir as mybir
from concourse import bass_isa
from concourse.bass_utils import run_bass_kernel_spmd

F32 = mybir.dt.float32
F16 = mybir.dt.float16
U8 = mybir.dt.uint8
I16 = mybir.dt.int16
OP = mybir.AluOpType
AX = mybir.AxisListType
ACT = mybir.ActivationFunctionType

B, S, D, H = 4, 4096, 4096, 1024
K = S // 2                  # 2048 selected per row
NCORES = 8
ROWS_PER_CORE = 2048        # (b, s) rows of x per core
NST = ROWS_PER_CORE // 128  # 16 seq tiles per core
NDC = D // 128              # 32 contraction chunks
TAB = K + 128               # gather table size (zero slot at index K)

_CACHED = {}


def _build():
    nc = bacc.Bacc("TRN2", target_bir_lowering=False, debug=False,
                   num_devices=NCORES)
    xs_d = nc.dram_tensor("xs", [ROWS_PER_CORE, D], F32, kind="ExternalInput")
    w1_d = nc.dram_tensor("w1", [D, H], F32, kind="ExternalInput")
    b1_d = nc.dram_tensor("b1", [H], F32, kind="ExternalInput")
    w2_d = nc.dram_tensor("w2", [H, 1], F32, kind="ExternalInput")
    b2_d = nc.dram_tensor("b2", [1], F32, kind="ExternalInput")
    mask_d = nc.dram_tensor("mask_row", [S], U8, kind="ExternalOutput")
    rw_d = nc.dram_tensor("rw_row", [S], F32, kind="ExternalOutput")

    with tile.TileContext(nc) as tc:
        with (
            tc.tile_pool(name="keep", bufs=1) as keep,
            tc.tile_pool(name="dram", bufs=1, space="DRAM") as dram,
        ):
            # ---------------- constants ----------------
            w2rep = keep.tile([128, H], F32)
            nc.sync.dma_start(
                w2rep[:],
                w2_d.ap().rearrange("h o -> (h o)").unsqueeze(0).broadcast_to([128, H]))
            b2col = keep.tile([128, 1], F32)
            nc.sync.dma_start(b2col[:], b2_d.ap().unsqueeze(0).broadcast_to([128, 1]))
            b1sb = keep.tile([1, H], F32)
            nc.sync.dma_start(b1sb[:], b1_d.ap().unsqueeze(0))
            b1h = keep.tile([1, H], F16)
            b1l = keep.tile([1, H], F16)
            nc.vector.tensor_copy(b1h[:], b1sb[:])
            nc.vector.tensor_tensor(b1l[:], b1sb[:], b1h[:], OP.subtract)
            onesrow = keep.tile([1, 128], F16)
            nc.vector.memset(onesrow[:], 1.0)

            iotasq = keep.tile([128, 128], F32)   # value = f - p
            nc.gpsimd.iota(iotasq[:], [[1, 128]], base=0, channel_multiplier=-1,
                           allow_small_or_imprecise_dtypes=True)
            ident16 = keep.tile([128, 128], F16)  # PE transpose identity
            nc.vector.tensor_scalar(ident16[:], iotasq[:], 0.0, None, OP.is_equal)
            lstrict = keep.tile([128, 128], F16)  # [p, f] = 1 if f > p
            nc.vector.tensor_scalar(lstrict[:], iotasq[:], 0.0, None, OP.is_gt)
            onescol = keep.tile([128, 1], F16)
            nc.vector.memset(onescol[:], 1.0)
            scores_sb = keep.tile([128, NST], F32)

            # ---------------- phase 1: scores = mlp(x) ----------------
            with (
                tc.tile_pool(name="w1pool", bufs=1) as w1pool,
                tc.tile_pool(name="xpool", bufs=2) as xpool,
                tc.tile_pool(name="xtpool", bufs=1) as xtpool,
                tc.tile_pool(name="epi", bufs=1) as epi,
                tc.tile_pool(name="pmm", bufs=2, space="PSUM") as pmm,
                tc.tile_pool(name="ptp", bufs=4, space="PSUM") as ptp,
            ):
                w1h = w1pool.tile([128, NDC * H], F16)
                w1l = w1pool.tile([128, NDC * H], F16)
                for dc in range(NDC):
                    wtmp = xpool.tile([128, H], F32, tag="x32")
                    nc.sync.dma_start(wtmp[:], w1_d.ap()[dc * 128:(dc + 1) * 128, :])
                    hview = w1h[:, dc * H:(dc + 1) * H]
                    nc.vector.tensor_copy(hview, wtmp[:])
                    nc.vector.tensor_tensor(w1l[:, dc * H:(dc + 1) * H],
                                            wtmp[:], hview, OP.subtract)

                for st in range(NST):
                    rows = slice(st * 128, (st + 1) * 128)
                    xh = xtpool.tile([128, D], F16, tag="xh")
                    xl = xtpool.tile([128, D], F16, tag="xl")
                    for half in range(2):
                        cols = slice(half * (D // 2), (half + 1) * (D // 2))
                        x32 = xpool.tile([128, D // 2], F32, tag="x32")
                        nc.sync.dma_start(x32[:], xs_d.ap()[rows, cols])
                        nc.vector.tensor_copy(xh[:, cols], x32[:])
                        nc.vector.tensor_tensor(xl[:, cols], x32[:], xh[:, cols],
                                                OP.subtract)
                    # transpose 128x128 blocks: xT[d, s]
                    xhT = xtpool.tile([128, D], F16, tag="xhT")
                    xlT = xtpool.tile([128, D], F16, tag="xlT")
                    for dc in range(NDC):
                        blk = slice(dc * 128, (dc + 1) * 128)
                        for src, dst in ((xh, xhT), (xl, xlT)):
                            pt = ptp.tile([128, 128], F16, tag="ptp")
                            nc.tensor.transpose(pt[:], src[:, blk], ident16[:])
                            nc.vector.tensor_copy(dst[:, blk], pt[:])

                    hpsum = pmm.tile([128, H], F32, tag="hpsum")
                    for dc in range(NDC):
                        blk = slice(dc * 128, (dc + 1) * 128)
                        first = dc == 0
                        for nh in range(2):
                            ncols = slice(nh * 512, (nh + 1) * 512)
                            wb = slice(dc * H + nh * 512, dc * H + (nh + 1) * 512)
                            nc.tensor.matmul(hpsum[:, ncols], xhT[:, blk],
                                             w1h[:, wb], start=first, stop=False)
                            nc.tensor.matmul(hpsum[:, ncols], xhT[:, blk],
                                             w1l[:, wb], start=False, stop=False)
                            nc.tensor.matmul(hpsum[:, ncols], xlT[:, blk],
                                             w1h[:, wb], start=False, stop=False)
                    # bias b1 (zero in practice, honored exactly)
                    for nh in range(2):
                        ncols = slice(nh * 512, (nh + 1) * 512)
                        nc.tensor.matmul(hpsum[:, ncols], onesrow[:],
                                         b1h[:, ncols], start=False, stop=False)
                        nc.tensor.matmul(hpsum[:, ncols], onesrow[:],
                                         b1l[:, ncols], start=False,
                                         stop=True)
                    # scores[:, st] = sum(relu(h) * w2)
                    escr = epi.tile([128, H], F32, tag="escr")
                    nc.vector.scalar_tensor_tensor(
                        escr[:], hpsum[:], 0.0, w2rep[:], OP.max, OP.mult,
                        accum_out=scores_sb[:, st:st + 1])
                nc.vector.tensor_scalar(scores_sb[:], scores_sb[:], b2col[:],
                                        None, OP.add)

            # ---------------- phase 1.5: pairwise allgather ----------------
            bounce_in = dram.tile([ROWS_PER_CORE], F32)
            bounce_pair = dram.tile([S], F32)
            nc.sync.dma_start(
                bounce_in[:].rearrange("(st p) -> st p", st=NST, p=128).transpose([1, 0]),
                scores_sb[:])
            nc.gpsimd.collective_compute(
                "AllGather", OP.bypass,
                replica_groups=[[0, 1], [2, 3], [4, 5], [6, 7]],
                ins=[bounce_in[:].opt()],
                outs=[bounce_pair[:].opt()],
            )

            # ---------------- phase 2: topk mask + scrambled softmax -------
            with (
                tc.tile_pool(name="p2", bufs=1) as p2,
                tc.tile_pool(name="p2s", bufs=2) as p2s,
                tc.tile_pool(name="pp2", bufs=2, space="PSUM") as pp2,
            ):
                iotaF = p2.tile([128, K], F32)   # 0..K-1 along free dim
                nc.gpsimd.iota(iotaF[:], [[1, K]], base=0, channel_multiplier=0,
                               allow_small_or_imprecise_dtypes=True)
                zrow = bounce_pair
                zB = p2.tile([128, 32], F32)     # z[128t + p] at [p, t]
                nc.sync.dma_start(
                    zB[:], zrow[:].rearrange("(t p) -> p t", t=32, p=128))
                zrep = p2.tile([128, S], F32)
                nc.sync.dma_start(
                    zrep[:], zrow[:].unsqueeze(0).broadcast_to([128, S]))

                # exact descending ranks: rank_s = #{u : z_u > z_s}
                ranksB = p2.tile([128, 32], F32)
                for t in range(32):
                    cscr = p2s.tile([128, S], mybir.dt.bfloat16, tag="cscr")
                    nc.vector.tensor_scalar(cscr[:], zrep[:], zB[:, t:t + 1],
                                            0.0, OP.is_gt, op1=OP.add,
                                            accum_out=ranksB[:, t:t + 1])

                maskf = p2.tile([128, 32], F32)
                nc.vector.tensor_scalar(maskf[:], ranksB[:], float(K), None,
                                        OP.is_lt)
                masku8 = p2.tile([128, 32], U8)
                nc.vector.tensor_copy(masku8[:], maskf[:])
                nc.sync.dma_start(
                    mask_d.ap().rearrange("(t p) -> p t", t=32, p=128), masku8[:])
                maskh = p2.tile([128, 32], F16)
                nc.vector.tensor_copy(maskh[:], maskf[:])

                # exclusive prefix sum of mask via triangular matmuls
                psPS = pp2.tile([128, 32], F32, tag="psPS")
                nc.tensor.matmul(psPS[:], lstrict[:], maskh[:], start=True,
                                 stop=False)
                csPS = pp2.tile([1, 32], F32, tag="csPS")
                nc.tensor.matmul(csPS[:], onescol[:], maskh[:], start=True,
                                 stop=True)
                cs = p2.tile([1, 32], F32)
                nc.vector.tensor_copy(cs[:], csPS[:])
                zero32 = p2.tile([1, 32], F32)
                nc.vector.memset(zero32[:], 0.0)
                incl = p2.tile([1, 32], F32)
                nc.vector.tensor_tensor_scan(incl[:], cs[:], zero32[:], 0.0,
                                             OP.add, OP.add)
                excl = p2.tile([1, 32], F16)
                nc.vector.tensor_tensor(excl[:], incl[:], cs[:], OP.subtract)
                nc.tensor.matmul(psPS[:], onesrow[:], excl[:], start=False,
                                 stop=True)
                psB = p2.tile([128, 32], F32)
                nc.vector.tensor_copy(psB[:], psPS[:])

                # softmax pieces: M = global max, E = exp(z - M), Z = sum(E*mask)
                zmax = p2.tile([128, 1], F32)
                nc.vector.tensor_reduce(zmax[:], zB[:], axis=AX.X, op=OP.max)
                Mcol = p2.tile([128, 1], F32)
                nc.gpsimd.partition_all_reduce(Mcol[:], zmax[:], channels=128,
                                               reduce_op=bass_isa.ReduceOp.max)
                negM = p2.tile([128, 1], F32)
                nc.vector.tensor_scalar(negM[:], Mcol[:], -1.0, None, OP.mult)
                Ef = p2.tile([128, 32], F32)
                nc.scalar.activation(Ef[:], zB[:], ACT.Exp, bias=negM[:])
                Emask = p2.tile([128, 32], F32)
                Zpart = p2.tile([128, 1], F32)
                nc.vector.scalar_tensor_tensor(Emask[:], Ef[:], 0.0, maskf[:],
                                               OP.add, OP.mult,
                                               accum_out=Zpart[:])
                Zcol = p2.tile([128, 1], F32)
                nc.gpsimd.partition_all_reduce(Zcol[:], Zpart[:], channels=128,
                                               reduce_op=bass_isa.ReduceOp.add)
                rZ = p2.tile([128, 1], F32)
                nc.vector.reciprocal(rZ[:], Zcol[:])

                # payload columns (E_s, 1) per s-chunk, fp16
                pay = p2.tile([128, 64], F32)
                nc.vector.memset(pay[:], 1.0)
                nc.vector.tensor_copy(
                    pay[:].rearrange("p (t two) -> p t two", t=32, two=2)[:, :, 0],
                    Ef[:])

                # permutation via one-hot matmuls: table[r] = (E_(r), count_r)
                # (each t is a self-contained start/stop set into a fresh PSUM
                # tile -- interleaved accumulation groups in one bank clobber
                # each other's has_written state -- then accumulate on DVE)
                tabsb = p2.tile([128, 32], F32)
                nc.vector.memset(tabsb[:], 0.0)
                for t in range(32):
                    oh = p2s.tile([128, K], F32, tag="oh")
                    nc.vector.tensor_scalar(oh[:], iotaF[:], ranksB[:, t:t + 1],
                                            None, OP.is_equal)
                    tps = pp2.tile([128, 32], F32, tag="tabPS")
                    for rc in range(16):
                        nc.tensor.matmul(
                            tps[:, 2 * rc:2 * rc + 2],
                            oh[:, rc * 128:(rc + 1) * 128],
                            pay[:, 2 * t:2 * t + 2],
                            start=True, stop=True)
                    nc.vector.tensor_tensor(tabsb[:], tabsb[:], tps[:], OP.add)
                tabv = tabsb[:].rearrange("p (rc two) -> p rc two", rc=16, two=2)
                sortE = p2.tile([128, 16], F32)
                cnt = p2.tile([128, 16], F32)
                nc.vector.tensor_copy(sortE[:], tabv[:, :, 0])
                nc.vector.tensor_copy(cnt[:], tabv[:, :, 1])

                # D = E/(max(cnt,1) * Z);  b = cnt > 0
                cmax = p2.tile([128, 16], F32)
                nc.vector.tensor_scalar(cmax[:], cnt[:], 1.0, None, OP.max)
                crec = p2.tile([128, 16], F32)
                nc.vector.reciprocal(crec[:], cmax[:])
                Dt = p2.tile([128, 16], F32)
                nc.vector.tensor_tensor(Dt[:], sortE[:], crec[:], OP.mult)
                Dv = p2.tile([128, 16], F32)
                nc.vector.tensor_scalar(Dv[:], Dt[:], rZ[:], None, OP.mult)
                bv = p2.tile([128, 16], F32)
                nc.vector.tensor_scalar(bv[:], cnt[:], 0.0, None, OP.is_gt)

                # round-trip to [1, K] layout for the backfill scan
                dD = dram.tile([K], F32)
                dB = dram.tile([K], F32)
                nc.sync.dma_start(
                    dD[:].rearrange("(rc m) -> m rc", rc=16, m=128), Dv[:])
                nc.sync.dma_start(
                    dB[:].rearrange("(rc m) -> m rc", rc=16, m=128), bv[:])
                Drow = p2.tile([1, K], F32)
                brow = p2.tile([1, K], F32)
                nc.sync.dma_start(Drow[:], dD[:].unsqueeze(0))
                nc.sync.dma_start(brow[:], dB[:].unsqueeze(0))
                onemb = p2.tile([1, K], F32)
                nc.vector.tensor_scalar(onemb[:], brow[:], -1.0, 1.0, OP.mult,
                                        op1=OP.add)
                wrow = p2.tile([1, K], F32)
                nc.vector.tensor_tensor_scan(wrow[:], onemb[:], Drow[:], 0.0,
                                             OP.mult, OP.add)

                # replicated gather table with zero slot at K
                dT = dram.tile([TAB], F32)
                zpad = p2.tile([1, TAB - K], F32)
                nc.vector.memset(zpad[:], 0.0)
                nc.sync.dma_start(dT[:][0:K].unsqueeze(0), wrow[:])
                nc.sync.dma_start(dT[:][K:TAB].unsqueeze(0), zpad[:])
                tabRep = p2.tile([128, TAB], F32)
                nc.sync.dma_start(tabRep[:],
                                  dT[:].unsqueeze(0).broadcast_to([128, TAB]))

                # idx = mask ? ps : K   (int16, wrapped layout for ap_gather)
                a1 = p2.tile([128, 32], F32)
                nc.vector.tensor_scalar(a1[:], psB[:], -float(K), None, OP.add)
                a2 = p2.tile([128, 32], F32)
                nc.vector.tensor_tensor(a2[:], a1[:], maskf[:], OP.mult)
                idxf = p2.tile([128, 32], F32)
                nc.vector.tensor_scalar(idxf[:], a2[:], float(K), None, OP.add)
                idx16 = p2.tile([128, 32], I16)
                nc.vector.tensor_copy(idx16[:], idxf[:])
                dI = dram.tile([S], I16)
                nc.sync.dma_start(
                    dI[:].rearrange("(t p) -> p t", t=32, p=128), idx16[:])
                idxW = p2.tile([128, 32], I16)
                for g in range(8):
                    nc.sync.dma_start(
                        idxW[16 * g:16 * (g + 1), :],
                        dI[:][512 * g:512 * (g + 1)]
                        .rearrange("(f m) -> f m", f=32, m=16).transpose([1, 0]))

                gout = p2.tile([128, 512], F32)
                nc.gpsimd.ap_gather(gout[:], tabRep[:], idxW[:], channels=128,
                                    num_elems=TAB, d=1, num_idxs=512)
                nc.sync.dma_start(
                    rw_d.ap().rearrange("(g f) -> g f", g=8, f=512),
                    gout[:].rearrange("(g m) f -> g m f", g=8, m=16)[:, 0, :])

    nc.finalize()
    return nc


def _get_nc():
    if "nc" not in _CACHED:
        _CACHED["nc"] = _build()
    return _CACHED["nc"]


def _get_runner():
    """Cached jitted SPMD executor -- the same PJRT path that
    bass_utils.run_bass_kernel_spmd takes under axon (bass2jax
    run_bass_via_pjrt), but with the traced/jitted callable cached so
    repeat kernel() calls skip retracing and recompilation."""
    if "runner" in _CACHED:
        return _CACHED["runner"]
    import jax
    from jax.experimental.shard_map import shard_map
    from jax.sharding import Mesh, PartitionSpec
    from concourse import bass2jax

    nc = _get_nc()
    bass2jax.install_neuronx_cc_hook()
    pname = nc.partition_id_tensor.name if nc.partition_id_tensor else None
    in_names, out_names, out_avals = [], [], []
    for alloc in nc.m.functions[0].allocations:
        if not isinstance(alloc, mybir.MemoryLocationSet):
            continue
        name = alloc.memorylocations[0].name
        if alloc.kind == "ExternalInput":
            if name != pname:
                in_names.append(name)
        elif alloc.kind == "ExternalOutput":
            assert alloc.tensor_shape is not None and alloc.dtype is not None
            out_names.append(name)
            out_avals.append(jax.core.ShapedArray(
                tuple(alloc.tensor_shape), mybir.dt.np(alloc.dtype)))
    n_params = len(in_names)
    all_in = tuple(in_names + out_names + ([pname] if pname else []))

    def _body(*args):
        operands = list(args)
        if pname is not None:
            operands.append(bass2jax.partition_id_tensor())
        outs = bass2jax._bass_exec_p.bind(
            *operands, out_avals=tuple(out_avals), in_names=all_in,
            out_names=tuple(out_names), lowering_input_output_aliases=(),
            sim_require_finite=True, sim_require_nnan=True, nc=nc)
        return tuple(outs)

    devices = jax.devices()[:NCORES]
    mesh = Mesh(np.asarray(devices), ("core",))
    donate = tuple(range(n_params, n_params + len(out_names)))
    sharded = jax.jit(
        shard_map(_body, mesh=mesh,
                  in_specs=(PartitionSpec("core"),) * (n_params + len(out_names)),
                  out_specs=(PartitionSpec("core"),) * len(out_names),
                  check_rep=False),
        donate_argnums=donate, keep_unused=True)
    _CACHED["runner"] = (sharded, in_names, out_names, out_avals)
    return _CACHED["runner"]


def _run(in_maps):
    import jax
    sharded, in_names, out_names, out_avals = _get_runner()
    concat_in = [np.concatenate([np.asarray(in_maps[c][n]) for c in range(NCORES)],
                                axis=0) for n in in_names]
    zeros = [np.zeros((NCORES * a.shape[0], *a.shape[1:]), a.dtype)
             for a in out_avals]
    out_arrs = sharded(*concat_in, *zeros)
    return [
        {n: np.asarray(out_arrs[i]).reshape(NCORES, *out_avals[i].shape)[c]
         for i, n in enumerate(out_names)}
        for c in range(NCORES)
    ]


def kernel(x, w1, b1, w2, b2):
    x = np.ascontiguousarray(np.asarray(x, dtype=np.float32))
    w1 = np.ascontiguousarray(np.asarray(w1, dtype=np.float32))
    b1 = np.ascontiguousarray(np.asarray(b1, dtype=np.float32))
    w2 = np.ascontiguousarray(np.asarray(w2, dtype=np.float32))
    b2 = np.ascontiguousarray(np.asarray(b2, dtype=np.float32))
    xf = x.reshape(B * S, D)

    in_maps = [
        {
            "xs": np.ascontiguousarray(
                xf[c * ROWS_PER_CORE:(c + 1) * ROWS_PER_CORE]),
            "w1": w1, "b1": b1, "w2": w2, "b2": b2,
        }
        for c in range(NCORES)
    ]
    results = _run(in_maps)
    mask = np.stack([results[2 * b]["mask_row"] for b in range(B)])
    rw = np.stack([results[2 * b]["rw_row"] for b in range(B)])
    return mask.astype(bool), rw.astype(np.float32)
